# revision 1
# baseline (speedup 1.0000x reference)
"""Trainium2 Bass kernel for nn_MultiHeadAttention_88854283419963 (TriAffine attention).

8 NeuronCores, SPMD.  The TriAffine contraction
    s[b,x,y,z,r] = sum_{i,k,j} xaug[b,x,i] mid[b,z,k] Wtri[i,k,j,r] yaug[b,y,j]
is factored k -> i -> j.  Wtri is sharded along j (48 j's per core); the
per-core partial s is ReduceScattered over x (16 x's per core), then each core
does the masked softmax over z, the alpha*text contraction, relu + Vw dot, an
AllGather of score chunks, and the replicated final combine with p_attn +
global min/max normalize + final softmax.

Bias row/col (index 384) of the augmented x/y are folded in as edge terms:
  - i=384 row   -> t_bias, added to u (broadcast over x) [j-sharded]
  - j=384 col   -> u_extra tiles, folded into the j-contraction as an extra
                   ones-weighted K row [scaled by 1/8: computed on all cores]
  - i=j=384     -> corner, added into u_extra [scaled by 1/8]
"""

import sys

sys.path.insert(0, "/opt/trn_rl_repo")
sys.path.insert(0, "/root/.axon_site/_ro/trn_rl_repo")

import math

import numpy as np

import concourse.bass as bass
import concourse.mybir as mybir
from concourse.masks import make_identity
from concourse.tile import TileContext
from bass_rust import ScopedClock

# ----------------------------------------------------------------------------
# Workaround: this container's walrus build rejects >1 sync-wait on the CTRL
# (Drain) instruction Tile emits at the kernel tail ("Too many sync wait
# commands").  Split the waits across single-wait NOPs instead.
# ----------------------------------------------------------------------------


def _patched_drain_and_barrier(self, tick_clock, wait_clock):
    probe = self.nc.sync.nop()
    wait_clock.add_sem_waits(probe.ins, ScopedClock({None: tick_clock.global_clock}))
    si = probe.ins.sync_info
    if si is not None and len(si.on_wait) > 1:
        waits = list(si.on_wait)
        probe.ins.sync_info = mybir.SyncInfo(
            on_wait=[waits[0]], on_update=list(si.on_update)
        )
        for w in waits[1:]:
            extra = self.nc.sync.nop()
            extra.ins.sync_info = mybir.SyncInfo(on_wait=[w], on_update=[])
    self.nc.sync.drain()
    self.nc.all_engine_barrier()
    assert self.sems is not None
    popped = self.nc._tile_sem_poison_stack.pop()
    assert popped is self._sem_poison
    self.nc.clear_and_free_semaphores(list(self.sems.allocated().values()))
    self.nc.all_engine_barrier()


TileContext._drain_and_barrier = _patched_drain_and_barrier

_NOPN = [0]


def _split_multiwaits(nc, limit=1):
    """walrus in this container accepts at most one sync-wait per instruction;
    move extra waits onto same-engine NoOps inserted just before."""
    for f in nc.m.functions:
        for blk in f.blocks:
            changed = False
            new = []
            for inst in blk.instructions:
                si = getattr(inst, "sync_info", None)
                if si is not None and len(si.on_wait) > limit:
                    ow = list(si.on_wait)
                    for w in ow[:-limit]:
                        _NOPN[0] += 1
                        nop = mybir.InstNoOp(name=f"mwsplit_{_NOPN[0]}", ins=[], outs=[])
                        nop.engine = inst.engine
                        nop.sync_info = mybir.SyncInfo(on_wait=[w], on_update=[])
                        new.append(nop)
                    inst.sync_info = mybir.SyncInfo(
                        on_wait=ow[-limit:], on_update=list(si.on_update)
                    )
                    changed = True
                new.append(inst)
            if changed:
                blk.instructions = new

# ----------------------------------------------------------------------------
B, L, D = 2, 128, 768
H, DK = 4, 192
HD, NC = 384, 2
CORES = 8
JC = HD // CORES          # 48
XL = L // CORES           # 16
NJR = 2 * JC              # 96
NJRE = NJR + 2            # + 2 bias-j columns
BL = B * L                # 256

F32 = mybir.dt.float32
F16 = mybir.dt.float16

DT_CHAIN = F32            # MM1/MM2 operand dtype
NP_CHAIN = np.float32
DT_STORE = F16            # u / R / MM3 / MM4 storage dtype
W_CHUNK = 4               # jr's per streamed W chunk

A = mybir.ActivationFunctionType
Alu = mybir.AluOpType
Ax = mybir.AxisListType


def build(debug=False):
    nc = bass.Bass(num_devices=CORES)

    # ---- inputs ----
    teT = nc.dram_tensor("teT", [128, 6, BL], F32, kind="ExternalInput")
    text16 = nc.dram_tensor("text16", [128, B, D], DT_STORE, kind="ExternalInput")
    mlp_in = {}
    for nm in ("h", "m"):
        mlp_in[nm] = (
            nc.dram_tensor(f"W{nm}1", [128, 6, HD], F32, kind="ExternalInput"),
            nc.dram_tensor(f"b{nm}1", [128, 3], F32, kind="ExternalInput"),
            nc.dram_tensor(f"W{nm}2", [128, 3, HD], F32, kind="ExternalInput"),
            nc.dram_tensor(f"b{nm}2", [128, 3], F32, kind="ExternalInput"),
        )
    Wt1 = nc.dram_tensor("Wt1", [128, 6, HD], F32, kind="ExternalInput")
    bt1 = nc.dram_tensor("bt1", [128, 3], F32, kind="ExternalInput")
    Wt2c = nc.dram_tensor("Wt2c", [128, 3, JC], F32, kind="ExternalInput")
    bt2c = nc.dram_tensor("bt2c", [JC, 1], F32, kind="ExternalInput")

    Wq = nc.dram_tensor("Wq", [128, 6, D], F32, kind="ExternalInput")
    bq = nc.dram_tensor("bq", [128, 6], F32, kind="ExternalInput")
    Wk = nc.dram_tensor("Wk", [128, 6, D], F32, kind="ExternalInput")
    bk = nc.dram_tensor("bk", [128, 6], F32, kind="ExternalInput")
    qryT = nc.dram_tensor("qryT", [128, 6, BL], F32, kind="ExternalInput")
    keyT = nc.dram_tensor("keyT", [128, 6, BL], F32, kind="ExternalInput")
    pmask = nc.dram_tensor("pmask", [128, B, L], F32, kind="ExternalInput")

    W1c = nc.dram_tensor("W1c", [NJRE, 3, 128, HD], DT_STORE, kind="ExternalInput")
    Wbi = nc.dram_tensor("Wbi", [128, 3, NJR], DT_STORE, kind="ExternalInput")
    Wcc = nc.dram_tensor("Wcc", [128, 3, 2], DT_STORE, kind="ExternalInput")

    m0 = nc.dram_tensor("m0", [128, XL, 128], F32, kind="ExternalInput")
    madd = nc.dram_tensor("madd", [128, XL, 128], F32, kind="ExternalInput")
    Vw_in = nc.dram_tensor("Vw", [2, D], F32, kind="ExternalInput")
    erow = nc.dram_tensor("erow", [2, 2, 128], DT_STORE, kind="ExternalInput")
    Vb_in = nc.dram_tensor("Vb", [2, 1], F32, kind="ExternalInput")

    out = nc.dram_tensor("out", [B, H, L, L], F32, kind="ExternalOutput")

    s_pre = nc.dram_tensor("s_pre", [CORES, B, NC, XL, L, L], F32)
    s_rs = nc.dram_tensor("s_rs", [B, NC, XL, L, L], F32)
    ag_in = nc.dram_tensor("ag_in", [B * NC * XL, L], F32)
    e3_dram = nc.dram_tensor("e3_dram", [2, BL], DT_STORE)
    rcp_dram = nc.dram_tensor("rcp_dram", [1, 1], F32)
    ag_out = nc.dram_tensor("ag_out", [CORES, B * NC * XL, L], F32, addr_space="Shared")

    dbg = {}
    if debug:
        dbg["headT"] = nc.dram_tensor("dbg_headT", [128, 3, BL], F32, kind="ExternalOutput")
        dbg["midT"] = nc.dram_tensor("dbg_midT", [128, 3, BL], F32, kind="ExternalOutput")
        dbg["tailc"] = nc.dram_tensor("dbg_tailc", [JC, BL], F32, kind="ExternalOutput")
        dbg["tbias"] = nc.dram_tensor("dbg_tbias", [NJR, BL], F32, kind="ExternalOutput")
        dbg["u"] = nc.dram_tensor("dbg_u", [B, 128, NJR, 128], F32, kind="ExternalOutput")
        dbg["uex"] = nc.dram_tensor("dbg_uex", [128, B * NC, 128], F32, kind="ExternalOutput")
        dbg["spre"] = nc.dram_tensor("dbg_spre", [CORES, B, NC, XL, L, L], F32, kind="ExternalOutput")
        dbg["srs"] = nc.dram_tensor("dbg_srs", [B, NC, XL, L, L], F32, kind="ExternalOutput")
        dbg["score"] = nc.dram_tensor("dbg_score", [CORES, B * NC * XL, L], F32, kind="ExternalOutput")
        dbg["pattn"] = nc.dram_tensor("dbg_pattn", [B, H, L, L], F32, kind="ExternalOutput")

    with TileContext(nc) as tc:
        with (
            tc.tile_pool(name="res", bufs=1) as res,
            tc.tile_pool(name="res16", bufs=1) as res16,
        ):
            ident16 = res16.tile([128, 128], DT_STORE)
            make_identity(nc, ident16)
            ident32 = res.tile([128, 128], F32)
            make_identity(nc, ident32)

            text_sb = res16.tile([128, B, D], DT_STORE)
            nc.sync.dma_start(text_sb[:], text16[:])
            vwb = res.tile([128, 2, D], F32)
            for r in range(2):
                nc.sync.dma_start(
                    vwb[:, r, :], Vw_in[r : r + 1, :].to_broadcast([128, D])
                )
            vbb = res.tile([128, 2], F32)
            for r in range(2):
                nc.sync.dma_start(
                    vbb[:, r : r + 1], Vb_in[r : r + 1, :].to_broadcast([128, 1])
                )
            m0_sb = res.tile([128, XL, 128], F32)
            nc.sync.dma_start(m0_sb[:], m0[:])
            madd_sb = res.tile([128, XL, 128], F32)
            nc.sync.dma_start(madd_sb[:], madd[:])

            headT = res.tile([128, 3, BL], DT_CHAIN, name="headT")
            midT = res.tile([128, 3, BL], DT_CHAIN, name="midT")
            tailc16 = res16.tile([JC, BL], DT_STORE, name="tailc16")
            pattn = res.tile([128, B * H, L], F32, name="pattn")
            score_sb = res.tile([128, B * NC * XL], F32, name="score_sb")

            midT16 = res16.tile([128, 3, BL], DT_STORE, name="midT16")
            tbias16 = res16.tile([NJR, BL], DT_STORE, name="tbias16")
            e3b = res16.tile([128, 2, BL], DT_STORE, name="e3b")
            u_sb = res16.tile([128, B, NJR, 128], DT_STORE, name="u_sb")
            uex = res16.tile([128, B * NC, 128], DT_STORE, name="uex")

            # ================= stage A: MLPs + p_attn =================
            with (
                tc.tile_pool(name="mlpw", bufs=1) as mlpw,
                tc.tile_pool(name="psA", bufs=3, space="PSUM") as psA,
                tc.tile_pool(name="tmpA", bufs=2) as tmpA,
                tc.tile_pool(name="qpkp", bufs=1) as qpkp,
            ):
                teT_sb = mlpw.tile([128, 6, BL], F32)
                nc.sync.dma_start(teT_sb[:], teT[:])

                # --- head / mid MLPs (full layer2) ---
                for nm in ("h", "m"):
                    W1d, b1d, W2d, b2d = mlp_in[nm]
                    w1 = mlpw.tile([128, 6, HD], F32, name="w1s")
                    nc.sync.dma_start(w1[:], W1d[:])
                    b1 = mlpw.tile([128, 3], F32, name="b1s")
                    nc.sync.dma_start(b1[:], b1d[:])
                    w2 = mlpw.tile([128, 3, HD], F32, name="w2s")
                    nc.sync.dma_start(w2[:], W2d[:])
                    b2 = mlpw.tile([128, 3], F32, name="b2s")
                    nc.sync.dma_start(b2[:], b2d[:])

                    h1 = tmpA.tile([128, 3, BL], F32, name="h1")
                    for mt in range(3):
                        ps = psA.tile([128, BL], F32, name="psA")
                        for ks in range(6):
                            nc.tensor.matmul(
                                ps[:], w1[:, ks, mt * 128 : (mt + 1) * 128],
                                teT_sb[:, ks, :], start=(ks == 0), stop=(ks == 5),
                            )
                        nc.scalar.activation(
                            h1[:, mt, :], ps[:], A.Relu, bias=b1[:, mt : mt + 1]
                        )
                    dst = headT if nm == "h" else midT
                    for mt in range(3):
                        ps = psA.tile([128, BL], F32, name="psA")
                        for ks in range(3):
                            nc.tensor.matmul(
                                ps[:], w2[:, ks, mt * 128 : (mt + 1) * 128],
                                h1[:, ks, :], start=(ks == 0), stop=(ks == 2),
                            )
                        nc.scalar.activation(
                            dst[:, mt, :], ps[:], A.Identity, bias=b2[:, mt : mt + 1]
                        )
                    if debug:
                        key = "headT" if nm == "h" else "midT"
                        nc.sync.dma_start(dbg[key][:], dst[:])

                for mt in range(3):
                    nc.vector.tensor_copy(midT16[:, mt, :], midT[:, mt, :])

                # --- tail MLP: full layer1, per-core 48-row layer2 ---
                w1 = mlpw.tile([128, 6, HD], F32, name="w1s")
                nc.sync.dma_start(w1[:], Wt1[:])
                b1 = mlpw.tile([128, 3], F32, name="b1s")
                nc.sync.dma_start(b1[:], bt1[:])
                w2c = mlpw.tile([128, 3, JC], F32, name="w2c")
                nc.sync.dma_start(w2c[:], Wt2c[:])
                b2c = mlpw.tile([JC, 1], F32, name="b2c")
                nc.sync.dma_start(b2c[:], bt2c[:])
                h1 = tmpA.tile([128, 3, BL], F32, name="h1")
                for mt in range(3):
                    ps = psA.tile([128, BL], F32, name="psA")
                    for ks in range(6):
                        nc.tensor.matmul(
                            ps[:], w1[:, ks, mt * 128 : (mt + 1) * 128],
                            teT_sb[:, ks, :], start=(ks == 0), stop=(ks == 5),
                        )
                    nc.scalar.activation(
                        h1[:, mt, :], ps[:], A.Relu, bias=b1[:, mt : mt + 1]
                    )
                pst = psA.tile([JC, BL], F32, name="psA")
                for ks in range(3):
                    nc.tensor.matmul(
                        pst[:], w2c[:, ks, :], h1[:, ks, :],
                        start=(ks == 0), stop=(ks == 2),
                    )
                nc.scalar.activation(tailc16[:], pst[:], A.Identity, bias=b2c[:])
                if debug:
                    d32 = tmpA.tile([JC, BL], F32, name="dtc")
                    nc.scalar.activation(d32[:], pst[:], A.Identity, bias=b2c[:])
                    nc.sync.dma_start(dbg["tailc"][:], d32[:])

                # --- p_attn ---
                wq_sb = mlpw.tile([128, 6, D], F32, name="wqk")
                nc.sync.dma_start(wq_sb[:], Wq[:])
                bq_sb = mlpw.tile([128, 6], F32, name="bqs")
                nc.sync.dma_start(bq_sb[:], bq[:])
                wk_sb = mlpw.tile([128, 6, D], F32, name="wqk")
                nc.sync.dma_start(wk_sb[:], Wk[:])
                bk_sb = mlpw.tile([128, 6], F32, name="bks")
                nc.sync.dma_start(bk_sb[:], bk[:])
                qT_sb = mlpw.tile([128, 6, BL], F32, name="qkT")
                nc.sync.dma_start(qT_sb[:], qryT[:])
                kT_sb = mlpw.tile([128, 6, BL], F32, name="qkT")
                nc.sync.dma_start(kT_sb[:], keyT[:])
                pm_sb = mlpw.tile([128, B, L], F32, name="pm")
                nc.sync.dma_start(pm_sb[:], pmask[:])

                qpT = qpkp.tile([128, 6, BL], F32, name="qpT")
                kpT = qpkp.tile([128, 6, BL], F32, name="kpT")
                for wmat, bvec, src, dst2 in (
                    (wq_sb, bq_sb, qT_sb, qpT),
                    (wk_sb, bk_sb, kT_sb, kpT),
                ):
                    for mt in range(6):
                        ps = psA.tile([128, BL], F32, name="psA")
                        for ks in range(6):
                            nc.tensor.matmul(
                                ps[:], wmat[:, ks, mt * 128 : (mt + 1) * 128],
                                src[:, ks, :], start=(ks == 0), stop=(ks == 5),
                            )
                        nc.scalar.activation(
                            dst2[:, mt, :], ps[:], A.Identity, bias=bvec[:, mt : mt + 1]
                        )

                inv_sqrt = 1.0 / math.sqrt(DK)
                for b in range(B):
                    for h in range(H):
                        ps = psA.tile([128, 128], F32, name="psA")
                        r0 = h * DK
                        segs = []
                        base = r0
                        while base < r0 + DK:
                            s_i, p0 = base // 128, base % 128
                            n = min(128 - p0, r0 + DK - base)
                            segs.append((s_i, p0, n))
                            base += n
                        for si, (s_i, p0, n) in enumerate(segs):
                            nc.tensor.matmul(
                                ps[:],
                                qpT[p0 : p0 + n, s_i, b * L : (b + 1) * L],
                                kpT[p0 : p0 + n, s_i, b * L : (b + 1) * L],
                                start=(si == 0), stop=(si == len(segs) - 1),
                            )
                        sc = tmpA.tile([128, 128], F32, name="scq")
                        nc.vector.scalar_tensor_tensor(
                            sc[:], ps[:], inv_sqrt, pm_sb[:, b, :], Alu.mult, Alu.add
                        )
                        mx = tmpA.tile([128, 1], F32, name="mxq")
                        nc.vector.tensor_reduce(mx[:], sc[:], Ax.X, Alu.max, negate=True)
                        esum = tmpA.tile([128, 1], F32, name="esq")
                        e = tmpA.tile([128, 128], F32, name="eq")
                        nc.scalar.activation(
                            e[:], sc[:], A.Exp, bias=mx[:], accum_out=esum[:]
                        )
                        rec = tmpA.tile([128, 1], F32, name="recq")
                        nc.vector.reciprocal(rec[:], esum[:])
                        nc.vector.tensor_scalar_mul(pattn[:, b * H + h, :], e[:], rec[:])
                if debug:
                    for b in range(B):
                        for h in range(H):
                            nc.sync.dma_start(dbg["pattn"][b, h], pattn[:, b * H + h, :])

            # ================= stage B: t_bias + corner =================
            with (
                tc.tile_pool(name="sbB", bufs=1) as sbB,
                tc.tile_pool(name="psB", bufs=2, space="PSUM") as psB,
            ):
                wbi_sb = sbB.tile([128, 3, NJR], DT_STORE)
                nc.sync.dma_start(wbi_sb[:], Wbi[:])
                wcc_sb = sbB.tile([128, 3, 2], DT_STORE)
                nc.sync.dma_start(wcc_sb[:], Wcc[:])

                ps = psB.tile([NJR, BL], F32, name="psTB")
                for ks in range(3):
                    nc.tensor.matmul(
                        ps[:], wbi_sb[:, ks, :], midT16[:, ks, :],
                        start=(ks == 0), stop=(ks == 2),
                    )
                nc.scalar.activation(tbias16[:], ps[:], A.Copy)
                if debug:
                    d32 = sbB.tile([NJR, BL], F32, name="dtb")
                    nc.vector.tensor_copy(d32[:], ps[:])
                    nc.sync.dma_start(dbg["tbias"][:], d32[:])

                psc = psB.tile([2, BL], F32, name="psCC")
                for ks in range(3):
                    nc.tensor.matmul(
                        psc[:], wcc_sb[:, ks, :], midT16[:, ks, :],
                        start=(ks == 0), stop=(ks == 2),
                    )
                e3 = sbB.tile([2, BL], DT_STORE, name="e3")
                nc.scalar.activation(e3[:], psc[:], A.Copy, scale=0.125)
                # broadcast each r-row across partitions (DRAM bounce: SBUF
                # source APs may not have a zero partition step)
                nc.sync.dma_start(e3_dram[:], e3[:])
                for r in range(2):
                    nc.sync.dma_start(
                        e3b[:, r, :], e3_dram[r : r + 1, :].to_broadcast([128, BL])
                    )

            # ================= stage C: jr loop (MM1 + MM2) =================
            with (
                tc.tile_pool(name="wchunk", bufs=2) as wchunk,
                tc.tile_pool(name="tbig", bufs=2) as tbigp,
                tc.tile_pool(name="psT", bufs=4, space="PSUM") as psT,
                tc.tile_pool(name="psU", bufs=2, space="PSUM") as psU,
            ):
                n_chunks = (NJRE + W_CHUNK - 1) // W_CHUNK
                for ch in range(n_chunks):
                    jr0 = ch * W_CHUNK
                    g = min(W_CHUNK, NJRE - jr0)
                    wt = wchunk.tile([128, 3, W_CHUNK, HD], DT_STORE, name="wt")
                    for s in range(3):
                        nc.sync.dma_start(
                            wt[:, s, :g, :],
                            W1c[jr0 : jr0 + g, s].rearrange("g k i -> k g i"),
                        )
                    # MM1 (f16): t_big[i, it, jl, (b z)]
                    t_big = tbigp.tile([128, 3, W_CHUNK, BL], DT_CHAIN, name="t_big")
                    for jl in range(g):
                        for it in range(3):
                            ps = psT.tile([128, BL], F32, name="psT")
                            for ks in range(3):
                                nc.tensor.matmul(
                                    ps[:],
                                    wt[:, ks, jl, it * 128 : (it + 1) * 128],
                                    midT16[:, ks, :],
                                    start=(ks == 0), stop=(ks == 2),
                                )
                            nc.scalar.activation(t_big[:, it, jl, :], ps[:], A.Copy)
                    # MM2 (f32r): N = g*128 <= 512 over (jl, z) for each b
                    for b in range(B):
                        psu = psU.tile([128, W_CHUNK * 128], F32, name="psU")
                        rhs_n = g * 128
                        for it in range(3):
                            nc.tensor.matmul(
                                psu[:, :rhs_n],
                                headT[:, it, b * L : (b + 1) * L],
                                t_big[:, it, :g, b * L : (b + 1) * L],
                                start=(it == 0), stop=(it == 2),
                            )
                        if jr0 < NJR:
                            nc.scalar.activation(
                                u_sb[:, b, jr0 : jr0 + g, :],
                                psu[:, :rhs_n].rearrange("p (g z) -> p g z", z=128),
                                A.Copy,
                            )
                        else:
                            for rr in range(g):
                                nc.scalar.activation(
                                    uex[:, b * NC + rr, :],
                                    psu[:, rr * 128 : (rr + 1) * 128],
                                    A.Copy, scale=0.125,
                                )
                                nc.vector.tensor_tensor(
                                    uex[:, b * NC + rr, :],
                                    uex[:, b * NC + rr, :],
                                    e3b[:, rr, b * L : (b + 1) * L],
                                    Alu.add,
                                )

            if debug:
                with tc.tile_pool(name="dbgu", bufs=2) as dbgu:
                    for b in range(B):
                        for jr in range(NJR):
                            d32 = dbgu.tile([128, 128], F32, name="du")
                            nc.vector.tensor_copy(d32[:], u_sb[:, b, jr, :])
                            nc.sync.dma_start(dbg["u"][b, :, jr, :], d32[:])
                    for q in range(B * NC):
                        d32 = dbgu.tile([128, 128], F32, name="du")
                        nc.vector.tensor_copy(d32[:], uex[:, q, :])
                        nc.sync.dma_start(dbg["uex"][:, q, :], d32[:])

            # ============ stage D: transpose u, fold E1, MM3, s out ============
            with (
                tc.tile_pool(name="lhs3", bufs=1) as lhs3p,
                tc.tile_pool(name="Rp", bufs=2) as Rp,
                tc.tile_pool(name="psTr", bufs=4, space="PSUM") as psTr,
                tc.tile_pool(name="psS3", bufs=2, space="PSUM") as psS3,
                tc.tile_pool(name="sstage", bufs=4) as sstage,
            ):
                lhs = {}
                for b in range(B):
                    for r in range(NC):
                        lt = lhs3p.tile([128, 128], DT_STORE, name=f"lhs_{b}_{r}")
                        nc.vector.memset(lt[:], 0.0)
                        # tail rows at partitions [r*48, r*48+48)
                        nc.sync.dma_start(
                            lt[r * JC : (r + 1) * JC, :],
                            tailc16[:, b * L : (b + 1) * L],
                        )
                        # ones/zeros rows 96..97 (32-aligned DMA; a 1-row
                        # memset at partition 97 fails BIR verification)
                        nc.sync.dma_start(lt[NJR : NJR + 2, :], erow[r])
                        lhs[(b, r)] = lt

                for b in range(B):
                    R = Rp.tile([128, 128, 128], DT_STORE, name="R")
                    # u_extra rows (96, 97) via partition-collapsing DMA
                    for rr in range(NC):
                        nc.sync.dma_start(
                            R[NJR + rr : NJR + rr + 1, :, :],
                            uex[:, b * NC + rr, :],
                        )
                    # transpose [x, jr] -> [jr, x] for each z
                    for z in range(128):
                        pst = psTr.tile([128, 128], DT_STORE, name="psTr")
                        nc.tensor.transpose(
                            pst[0:NJR, :], u_sb[:, b, :, z], ident16[:]
                        )
                        nc.vector.tensor_copy(R[0:NJR, :, z], pst[0:NJR, :])
                    # fold E1: R[j] += t_bias[j, z] broadcast over x
                    nc.vector.tensor_tensor(
                        R[0:NJR, :, :],
                        R[0:NJR, :, :],
                        tbias16[:, b * L : (b + 1) * L][:, None, :].broadcast_to(
                            [NJR, 128, 128]
                        ),
                        Alu.add,
                    )
                    # MM3: s[y, (x,z)] per r, 512-wide chunks
                    for r in range(NC):
                        for chk in range(32):
                            x0 = chk * 4
                            ps = psS3.tile([128, 512], F32, name="psS3")
                            nc.tensor.matmul(
                                ps[:],
                                lhs[(b, r)][0 : NJRE, :],
                                R[0:NJRE, x0 : x0 + 4, :],
                                start=True, stop=True,
                            )
                            st = sstage.tile([128, 512], F32, name="st")
                            nc.vector.tensor_copy(st[:], ps[:])
                            nc.sync.dma_start(
                                s_pre[x0 // XL, b, r, x0 % XL : x0 % XL + 4, :, :]
                                .rearrange("x y z -> y x z"),
                                st[:].rearrange("y (x z) -> y x z", z=128),
                            )

            # ================= stage E: ReduceScatter =================
            nc.gpsimd.collective_compute(
                "ReduceScatter",
                Alu.add,
                replica_groups=[list(range(CORES))],
                ins=[s_pre[:]],
                outs=[s_rs[:]],
            )
            if debug:
                nc.sync.dma_start(dbg["spre"][:], s_pre[:])
                nc.sync.dma_start(dbg["srs"][:], s_rs[:])

            # ============ stage F: softmax over z + MM4 + score ============
            with (
                tc.tile_pool(name="postp", bufs=4) as postp,
                tc.tile_pool(name="post16", bufs=4) as post16,
                tc.tile_pool(name="psE", bufs=2, space="PSUM") as psE,
                tc.tile_pool(name="ps4", bufs=2, space="PSUM") as ps4p,
            ):
                for b in range(B):
                    for xl in range(XL):
                        for r in range(NC):
                            s_t = postp.tile([128, 128], F32, name="s_t")
                            nc.sync.dma_start(s_t[:], s_rs[b, r, xl])
                            sm = postp.tile([128, 128], F32, name="sm")
                            nc.vector.tensor_tensor(
                                sm[:], s_t[:], m0_sb[:, xl, :], Alu.mult
                            )
                            nc.vector.tensor_tensor(
                                sm[:], sm[:], madd_sb[:, xl, :], Alu.add
                            )
                            mx = postp.tile([128, 1], F32, name="mx")
                            nc.vector.tensor_reduce(
                                mx[:], sm[:], Ax.X, Alu.max, negate=True
                            )
                            e = postp.tile([128, 128], F32, name="e")
                            esum = postp.tile([128, 1], F32, name="esum")
                            nc.scalar.activation(
                                e[:], sm[:], A.Exp, bias=mx[:], accum_out=esum[:]
                            )
                            pse = psE.tile([128, 128], F32, name="psE")
                            nc.tensor.transpose(pse[:], e[:], ident32[:])
                            eT = post16.tile([128, 128], DT_STORE, name="eT")
                            nc.scalar.activation(eT[:], pse[:], A.Copy)
                            ps4 = ps4p.tile([128, D], F32, name="ps4")
                            nc.tensor.matmul(
                                ps4[:, 0:512], eT[:], text_sb[:, b, 0:512],
                                start=True, stop=True,
                            )
                            nc.tensor.matmul(
                                ps4[:, 512:768], eT[:], text_sb[:, b, 512:768],
                                start=True, stop=True,
                            )
                            junk = post16.tile([128, D], DT_STORE, name="junk")
                            acc = postp.tile([128, 1], F32, name="acc")
                            nc.vector.scalar_tensor_tensor(
                                junk[:], ps4[:], 0.0, vwb[:, r, :],
                                Alu.max, Alu.mult, accum_out=acc[:],
                            )
                            rec = postp.tile([128, 1], F32, name="rec")
                            nc.vector.reciprocal(rec[:], esum[:])
                            col = (b * NC + r) * XL + xl
                            nc.vector.tensor_scalar(
                                score_sb[:, col : col + 1], acc[:],
                                rec[:], vbb[:, r : r + 1], Alu.mult, Alu.add,
                            )

                # transpose scores -> [64, 128] and AllGather
                pse = psE.tile([128, 128], F32, name="psE")
                nc.tensor.transpose(
                    pse[0 : B * NC * XL, :], score_sb[:], ident32[:]
                )
                sc_t = postp.tile([B * NC * XL, 128], F32, name="sc_t")
                nc.vector.tensor_copy(sc_t[:], pse[0 : B * NC * XL, :])
                nc.sync.dma_start(ag_in[:], sc_t[:])

            nc.gpsimd.collective_compute(
                "AllGather",
                Alu.bypass,
                replica_groups=[list(range(CORES))],
                ins=[ag_in[:]],
                outs=[ag_out[:]],
            )
            if debug:
                nc.sync.dma_start(dbg["score"][:], ag_out[:])

            # ============ stage G: final combine (replicated) ============
            with (
                tc.tile_pool(name="finp", bufs=4) as finp,
                tc.tile_pool(name="psF", bufs=2, space="PSUM") as psF,
            ):
                combs = {}
                mm = finp.tile([128, 2], F32, name="mm")  # col0 max, col1 -min
                first = True
                for b in range(B):
                    for h in range(H):
                        # Reference does score4.reshape(B, H, L, L) -- a raw
                        # memory reinterpretation.  comb[b,h,i,j] =
                        # p_attn[b,h,i,j] + score[b, h*32+i//4,
                        # 32*(i%4)+j//4, j%4]  (0 for j%4 >= NC).
                        scx = finp.tile([128, 128], F32, name="scx")
                        nc.vector.memset(scx[:], 0.0)
                        scx_v = scx[:].rearrange("p (j1 j2) -> p j1 j2", j2=4)
                        for j2 in range(NC):
                            for i1h in range(2):
                                src_ap = ag_out[
                                    h * 2 + i1h,
                                    (b * NC + j2) * XL : (b * NC + j2) * XL + XL,
                                    :,
                                ].rearrange("q (i2 j1) -> q i2 j1", i2=4)
                                nc.sync.dma_start(
                                    scx_v[i1h * 64 : (i1h + 1) * 64, :, j2],
                                    src_ap,
                                )
                        comb = finp.tile([128, 128], F32, name=f"comb_{b}_{h}")
                        nc.vector.tensor_tensor(
                            comb[:], pattn[:, b * H + h, :], scx[:], Alu.add
                        )
                        combs[(b, h)] = comb
                        if first:
                            nc.vector.tensor_reduce(
                                mm[:, 0:1], comb[:], Ax.X, Alu.max
                            )
                            nc.vector.tensor_reduce(
                                mm[:, 1:2], comb[:], Ax.X, Alu.min, negate=True
                            )
                            first = False
                        else:
                            t2 = finp.tile([128, 2], F32, name="t2")
                            nc.vector.tensor_reduce(t2[:, 0:1], comb[:], Ax.X, Alu.max)
                            nc.vector.tensor_reduce(
                                t2[:, 1:2], comb[:], Ax.X, Alu.min, negate=True
                            )
                            # col0 = max, col1 = -min: both combine via max
                            nc.vector.tensor_tensor(mm[:], mm[:], t2[:], Alu.max)
                # cross-partition: transpose [128, 2] -> [2, 128]
                psf = psF.tile([128, 128], F32, name="psF")
                nc.tensor.transpose(psf[0:2, :], mm[:], ident32[:])
                hilo = finp.tile([2, 128], F32, name="hilo")
                nc.vector.tensor_copy(hilo[:], psf[0:2, :])
                # rows: [per-part maxes; per-part -mins] -> [2,1] via max
                hl2 = finp.tile([2, 1], F32, name="hl2")
                nc.vector.tensor_reduce(hl2[:], hilo[:], Ax.X, Alu.max)
                # hi - lo = hl2[0] + hl2[1]: collapse partitions via DMA
                hl_dram = nc.dram_tensor(f"hl_dram", [2, 1], F32)
                nc.sync.dma_start(hl_dram[:], hl2[:])
                hlrow = finp.tile([1, 2], F32, name="hlrow")
                nc.sync.dma_start(hlrow[:], hl_dram[:])
                rng = finp.tile([1, 1], F32, name="rng")
                nc.vector.tensor_reduce(rng[:], hlrow[:], Ax.X, Alu.add)
                rcp1 = finp.tile([1, 1], F32, name="rcp1")
                nc.vector.reciprocal(rcp1[:], rng[:])
                rcpb = finp.tile([128, 1], F32, name="rcpb")
                nc.sync.dma_start(rcp_dram[:], rcp1[:])
                nc.sync.dma_start(rcpb[:], rcp_dram[0:1, :].to_broadcast([128, 1]))

                for b in range(B):
                    for h in range(H):
                        comb = combs[(b, h)]
                        # softmax over y of comb * rcp  (shift by lo*rcp is a
                        # per-row constant -> softmax-invariant)
                        nrm = finp.tile([128, 128], F32, name="nrm")
                        nc.vector.tensor_scalar_mul(nrm[:], comb[:], rcpb[:])
                        mx = finp.tile([128, 1], F32, name="mxf")
                        nc.vector.tensor_reduce(
                            mx[:], nrm[:], Ax.X, Alu.max, negate=True
                        )
                        ef = finp.tile([128, 128], F32, name="ef")
                        esum = finp.tile([128, 1], F32, name="esf")
                        nc.scalar.activation(
                            ef[:], nrm[:], A.Exp, bias=mx[:], accum_out=esum[:]
                        )
                        rec = finp.tile([128, 1], F32, name="recf")
                        nc.vector.reciprocal(rec[:], esum[:])
                        of = finp.tile([128, 128], F32, name="of")
                        nc.vector.tensor_scalar_mul(of[:], ef[:], rec[:])
                        nc.sync.dma_start(out[b, h], of[:])

    _split_multiwaits(nc)
    return nc


# ----------------------------------------------------------------------------
# Host-side input preparation (per core)
# ----------------------------------------------------------------------------


def prep_inputs(inputs):
    """inputs: dict of full numpy arrays as produced by setup_inputs().
    Returns in_maps: list of per-core dicts."""
    f32 = np.float32
    te = np.ascontiguousarray(inputs["text_embeddings"], dtype=f32)  # [B, L, D]
    query = np.ascontiguousarray(inputs["query"], dtype=f32)
    key = np.ascontiguousarray(inputs["key"], dtype=f32)
    mask = inputs["mask"]
    Wtri = np.ascontiguousarray(inputs["Wtri"], dtype=f32)  # [385, 384, 385, 2]

    def kt(a, s):  # [K, M] -> [128, K//128, M]
        K, M = a.shape
        assert K == s * 128
        return np.ascontiguousarray(a.reshape(s, 128, M).transpose(1, 0, 2))

    def rowsT(a):  # [B, L, D] -> [128, D//128, B*L] transposed k-tiled
        Dm = a.shape[-1]
        flat = a.reshape(-1, Dm).T  # [D, B*L]
        return np.ascontiguousarray(
            flat.reshape(Dm // 128, 128, flat.shape[1]).transpose(1, 0, 2)
        )

    def bias_t(b, s):  # [s*128] -> [128, s]
        return np.ascontiguousarray(b.reshape(s, 128).T)

    common = {
        "teT": rowsT(te),
        "text16": np.ascontiguousarray(te.transpose(1, 0, 2)).astype(np.float16),
        "Wh1": kt(inputs["Wh1"].astype(f32), 6),
        "bh1": bias_t(inputs["bh1"].astype(f32), 3),
        "Wh2": kt(inputs["Wh2"].astype(f32), 3),
        "bh2": bias_t(inputs["bh2"].astype(f32), 3),
        "Wm1": kt(inputs["Wm1"].astype(f32), 6),
        "bm1": bias_t(inputs["bm1"].astype(f32), 3),
        "Wm2": kt(inputs["Wm2"].astype(f32), 3),
        "bm2": bias_t(inputs["bm2"].astype(f32), 3),
        "Wt1": kt(inputs["Wt1"].astype(f32), 6),
        "bt1": bias_t(inputs["bt1"].astype(f32), 3),
        "Wq": kt(inputs["Wq"].astype(f32), 6),
        "bq": bias_t(inputs["bq"].astype(f32), 6),
        "Wk": kt(inputs["Wk"].astype(f32), 6),
        "bk": bias_t(inputs["bk"].astype(f32), 6),
        "qryT": rowsT(query),
        "keyT": rowsT(key),
        "pmask": np.ascontiguousarray(
            np.where(mask == 0, np.float32(-1e9), np.float32(0.0)).transpose(1, 0, 2)
        ),
        "Vw": inputs["Vw"].astype(f32).reshape(2, D),
        "erow": np.stack([
            np.stack([np.ones(128), np.zeros(128)]),
            np.stack([np.zeros(128), np.ones(128)]),
        ]).astype(np.float16),
        "Vb": inputs["Vb"].astype(f32).reshape(2, 1),
    }

    # corner [128, 3, 2]
    wcc = Wtri[384, :, 384, :]  # [384, 2]
    common["Wcc"] = np.ascontiguousarray(
        wcc.reshape(3, 128, 2).transpose(1, 0, 2)
    ).astype(np.float16)

    idx = np.arange(L)
    in_maps = []
    for c in range(CORES):
        m = dict(common)
        j0 = c * JC
        # W core slice -> [NJRE, 3, 128, 384]; jr = r*48 + jj
        blk = Wtri[:HD, :, j0 : j0 + JC, :]  # [384 i, 384 k, 48 j, 2 r]
        w1c = np.empty((NJRE, 3, 128, HD), dtype=np.float16)
        t = blk.transpose(3, 2, 1, 0)  # [r, j, k, i]
        w1c[:NJR] = t.reshape(NJR, 3, 128, HD)
        bj = Wtri[:HD, :, 384, :]  # [384 i, 384 k, 2 r]
        for r in range(2):
            w1c[NJR + r] = bj[:, :, r].T.reshape(3, 128, HD)
        m["W1c"] = np.ascontiguousarray(w1c)

        # bias-i rows: [128, 3, 96]
        bi = Wtri[384, :, j0 : j0 + JC, :]  # [384 k, 48 j, 2 r]
        tmp = bi.transpose(0, 2, 1).reshape(HD, NJR)  # [k, jr]
        m["Wbi"] = np.ascontiguousarray(
            tmp.reshape(3, 128, NJR).transpose(1, 0, 2)
        ).astype(np.float16)

        # per-core tail layer-2 slice
        m["Wt2c"] = np.ascontiguousarray(
            inputs["Wt2"].astype(f32)[:, j0 : j0 + JC].reshape(3, 128, JC)
            .transpose(1, 0, 2)
        )
        m["bt2c"] = np.ascontiguousarray(
            inputs["bt2"].astype(f32)[j0 : j0 + JC].reshape(JC, 1)
        )

        # softmax-z masks for this core's x chunk: [y(128), xl, z]
        xs = c * XL + np.arange(XL)
        zz = idx[None, None, :]
        yy = idx[:, None, None]
        xx = xs[None, :, None]
        bad = (zz > yy) | (zz < xx)  # [y, xl, z]
        m["m0"] = np.ascontiguousarray(np.where(bad, 0.0, 1.0).astype(f32))
        m["madd"] = np.ascontiguousarray(np.where(bad, -1e6, 0.0).astype(f32))
        in_maps.append(m)
    return in_maps


_CACHE = {}


def _get_built(debug=False):
    key = ("nc", debug)
    if key not in _CACHE:
        _CACHE[key] = build(debug=debug)
    return _CACHE[key]


def run(inputs, debug=False, trace=False):
    from concourse.bass_utils import run_bass_kernel_spmd

    nc = _get_built(debug=debug)
    in_maps = prep_inputs(inputs)
    res = run_bass_kernel_spmd(
        nc, in_maps, list(range(CORES)), trace=trace
    )
    return res


def kernel(**inputs):
    res = run(inputs, debug=False)
    return np.ascontiguousarray(res.results[0]["out"])


if __name__ == "__main__":
    nc = build(debug=False)
    print("build OK")



# revision 19
# speedup vs baseline: 2.1569x; 2.1569x over previous
"""Trainium2 Bass kernel for nn_MultiHeadAttention_88854283419963 (TriAffine attention).

8 NeuronCores, SPMD.  The TriAffine contraction
    s[b,x,y,z,r] = sum_{i,k,j} xaug[b,x,i] mid[b,z,k] Wtri[i,k,j,r] yaug[b,y,j]
is factored k -> i -> j.  Wtri is sharded along j (48 j's per core, jr = r*48+jj
r-major, plus 2 bias-j columns scaled 1/8 computed on all cores).

Per core: MM1 (k-contraction, W stationary) -> t[i, jr, bz]; MM2 (i-contraction,
head stationary) -> u[x, jr, b, z], staged and written corner-turned into an
AllToAll input laid out [dest_core, jr, b, xl, z].  Two AllToAlls (split by jr
so the first overlaps the tail of MM1/MM2) give each core the full 8x98 jr rows
for its own 16-x slice.  MM3 (j-contraction, tail lhs per jr-chunk) then
produces the full s[y, xl, z] for this core's x chunk directly in PSUM -- no
ReduceScatter of the 33 MB s tensor, no PE transposes of u.
t_bias (i=384 row of Wtri) is folded into the gathered R tiles; the j=384 bias
column rides along as u rows 96..97, summed via ones-rows in the MM3 lhs.

Stage F: masked softmax over z per (b,r,xl) tile + alpha@text (MM4) + relu.Vw
-> score.  Scores are written into the final comb layout (score4.reshape(B,H,L,L)
memory reinterpretation) before a 64KB AllGather, so the replicated final
combine reads two contiguous 32KB blocks per (b,h).
"""

import sys

sys.path.insert(0, "/opt/trn_rl_repo")
sys.path.insert(0, "/root/.axon_site/_ro/trn_rl_repo")

import math

import numpy as np

import concourse.bass as bass
import concourse.mybir as mybir
from concourse.masks import make_identity
from concourse.tile import TileContext
from bass_rust import ScopedClock

# ----------------------------------------------------------------------------
# Workaround: this container's walrus build rejects >1 sync-wait on the CTRL
# (Drain) instruction Tile emits at the kernel tail ("Too many sync wait
# commands").  Split the waits across single-wait NOPs instead.
# ----------------------------------------------------------------------------


def _patched_drain_and_barrier(self, tick_clock, wait_clock):
    probe = self.nc.sync.nop()
    wait_clock.add_sem_waits(probe.ins, ScopedClock({None: tick_clock.global_clock}))
    si = probe.ins.sync_info
    if si is not None and len(si.on_wait) > 1:
        waits = list(si.on_wait)
        probe.ins.sync_info = mybir.SyncInfo(
            on_wait=[waits[0]], on_update=list(si.on_update)
        )
        for w in waits[1:]:
            extra = self.nc.sync.nop()
            extra.ins.sync_info = mybir.SyncInfo(on_wait=[w], on_update=[])
    self.nc.sync.drain()
    self.nc.all_engine_barrier()
    assert self.sems is not None
    popped = self.nc._tile_sem_poison_stack.pop()
    assert popped is self._sem_poison
    self.nc.clear_and_free_semaphores(list(self.sems.allocated().values()))
    self.nc.all_engine_barrier()


TileContext._drain_and_barrier = _patched_drain_and_barrier

_NOPN = [0]


def _split_multiwaits(nc, limit=1):
    """walrus in this container accepts at most one sync-wait per instruction;
    move extra waits onto same-engine NoOps inserted just before."""
    for f in nc.m.functions:
        for blk in f.blocks:
            changed = False
            new = []
            for inst in blk.instructions:
                si = getattr(inst, "sync_info", None)
                if si is not None and len(si.on_wait) > limit:
                    ow = list(si.on_wait)
                    for w in ow[:-limit]:
                        _NOPN[0] += 1
                        nop = mybir.InstNoOp(name=f"mwsplit_{_NOPN[0]}", ins=[], outs=[])
                        nop.engine = inst.engine
                        nop.sync_info = mybir.SyncInfo(on_wait=[w], on_update=[])
                        new.append(nop)
                    inst.sync_info = mybir.SyncInfo(
                        on_wait=ow[-limit:], on_update=list(si.on_update)
                    )
                    changed = True
                new.append(inst)
            if changed:
                blk.instructions = new

# ----------------------------------------------------------------------------
B, L, D = 2, 128, 768
H, DK = 4, 192
HD, NC = 384, 2
CORES = 8
JC = HD // CORES          # 48
XL = L // CORES           # 16
NJR = 2 * JC              # 96
NJRE = NJR + 2            # + 2 bias-j columns
BL = B * L                # 256

F32 = mybir.dt.float32
F16 = mybir.dt.float16

W_CHUNK = 4               # jr's per streamed W chunk
JR_A = 48                 # jr rows in first AllToAll (chunks 0..11)
JR_B = NJRE - JR_A        # 50

A = mybir.ActivationFunctionType
Alu = mybir.AluOpType
Ax = mybir.AxisListType


def build(debug=False):
    nc = bass.Bass(num_devices=CORES)

    # ---- inputs ----
    teT16 = nc.dram_tensor("teT16", [128, 6, BL], F16, kind="ExternalInput")
    text16 = nc.dram_tensor("text16", [128, B, D], F16, kind="ExternalInput")
    mlp1 = {}
    for nm in ("h", "m", "t"):
        mlp1[nm] = (
            nc.dram_tensor(f"W{nm}1", [128, 6, HD], F16, kind="ExternalInput"),
            nc.dram_tensor(f"b{nm}1", [128, 3], F32, kind="ExternalInput"),
        )
    Wh2 = nc.dram_tensor("Wh2", [128, 3, HD], F16, kind="ExternalInput")
    bh2 = nc.dram_tensor("bh2", [128, 3], F32, kind="ExternalInput")
    Wm2 = nc.dram_tensor("Wm2", [128, 3, HD], F16, kind="ExternalInput")
    bm2 = nc.dram_tensor("bm2", [128, 3], F32, kind="ExternalInput")
    # tail layer-2 weights in per-(chunk, r) lhs layout: cols j' = r*48+jj hold
    # Wt2[:, 48c+jj], all other cols zero; ltb supplies bias + the ones-rows
    Wt2b = nc.dram_tensor("Wt2b", [128, 3, CORES, NC, NJRE], F16, kind="ExternalInput")
    ltb_in = nc.dram_tensor("ltb", [NJRE, CORES, NC], F32, kind="ExternalInput")

    Wq = nc.dram_tensor("Wq", [128, 6, D], F16, kind="ExternalInput")
    bq = nc.dram_tensor("bq", [128, 6], F32, kind="ExternalInput")
    Wk = nc.dram_tensor("Wk", [128, 6, D], F16, kind="ExternalInput")
    bk = nc.dram_tensor("bk", [128, 6], F32, kind="ExternalInput")
    qryT = nc.dram_tensor("qryT", [128, 6, BL], F16, kind="ExternalInput")
    keyT = nc.dram_tensor("keyT", [128, 6, BL], F16, kind="ExternalInput")
    pmask = nc.dram_tensor("pmask", [128, B, L], F32, kind="ExternalInput")

    W1c = nc.dram_tensor("W1c", [NJRE, 3, 128, HD], F16, kind="ExternalInput")
    Wbi = nc.dram_tensor("Wbi", [128, 3, CORES, NJR], F16, kind="ExternalInput")
    Wcc = nc.dram_tensor("Wcc", [128, 3, 2], F16, kind="ExternalInput")

    m0 = nc.dram_tensor("m0", [128, XL, 128], F32, kind="ExternalInput")
    madd = nc.dram_tensor("madd", [128, XL, 128], F32, kind="ExternalInput")
    Vw_in = nc.dram_tensor("Vw", [2, D], F32, kind="ExternalInput")
    Vb_in = nc.dram_tensor("Vb", [2, 1], F32, kind="ExternalInput")

    out = nc.dram_tensor("out", [B, H, L, L], F32, kind="ExternalOutput")

    # DRAM scratch / collective buffers (A2A blocks: [dest, xl, jr, b, z])
    ua_in_a = nc.dram_tensor("ua_in_a", [CORES, XL, JR_A, B, 128], F16)
    ua_in_b = nc.dram_tensor("ua_in_b", [CORES, XL, JR_B, B, 128], F16)
    ua_out_a = nc.dram_tensor("ua_out_a", [CORES, XL, JR_A, B, 128], F16)
    ua_out_b = nc.dram_tensor("ua_out_b", [CORES, XL, JR_B, B, 128], F16)
    ag_in = nc.dram_tensor("ag_in", [B, 64, 128], F32)
    ag_sc = nc.dram_tensor("ag_sc", [CORES, B, 64, 128], F32, addr_space="Shared")
    e3_dram = nc.dram_tensor("e3_dram", [2, BL], F16)
    rcp_dram = nc.dram_tensor("rcp_dram", [1, 1], F32)
    hl_dram = nc.dram_tensor("hl_dram", [2, 1], F32)

    dbg = {}
    if debug:
        dbg["headT"] = nc.dram_tensor("dbg_headT", [128, 3, BL], F32, kind="ExternalOutput")
        dbg["midT"] = nc.dram_tensor("dbg_midT", [128, 3, BL], F32, kind="ExternalOutput")
        dbg["tb"] = nc.dram_tensor("dbg_tb", [NJR, CORES, B, 128], F32, kind="ExternalOutput")
        dbg["lt"] = nc.dram_tensor("dbg_lt", [CORES, B, NC, NJRE, 128], F32, kind="ExternalOutput")
        dbg["ua"] = nc.dram_tensor("dbg_ua", [CORES, XL, JR_A, B, 128], F16, kind="ExternalOutput")
        dbg["ub"] = nc.dram_tensor("dbg_ub", [CORES, XL, JR_B, B, 128], F16, kind="ExternalOutput")
        dbg["R"] = nc.dram_tensor("dbg_R", [NJRE, CORES, XL, B, 128], F16, kind="ExternalOutput")
        dbg["s"] = nc.dram_tensor("dbg_s", [B, NC, 128, XL, 128], F32, kind="ExternalOutput")
        dbg["score"] = nc.dram_tensor("dbg_score", [128, B * 4 * XL], F32, kind="ExternalOutput")
        dbg["pattn"] = nc.dram_tensor("dbg_pattn", [B, H, L, L], F32, kind="ExternalOutput")

    with TileContext(nc) as tc:
        with (
            tc.tile_pool(name="res", bufs=1) as res,
            tc.tile_pool(name="res16", bufs=1) as res16,
        ):
            ident16 = res16.tile([128, 128], F16)
            make_identity(nc, ident16)
            ident32 = res.tile([128, 128], F32)
            make_identity(nc, ident32)

            text_sb = res16.tile([128, B, D], F16)
            nc.sync.dma_start(text_sb[:], text16[:])
            vwb = res.tile([128, 2, D], F32)
            for r in range(2):
                nc.sync.dma_start(
                    vwb[:, r, :], Vw_in[r : r + 1, :].to_broadcast([128, D])
                )
            vbb = res.tile([128, 2], F32)
            for r in range(2):
                nc.sync.dma_start(
                    vbb[:, r : r + 1], Vb_in[r : r + 1, :].to_broadcast([128, 1])
                )
            m0_sb = res.tile([128, XL, 128], F32)
            nc.sync.dma_start(m0_sb[:], m0[:])
            madd_sb = res.tile([128, XL, 128], F32)
            nc.sync.dma_start(madd_sb[:], madd[:])

            headT16 = res16.tile([128, 3, BL], F16, name="headT16")
            midT16 = res16.tile([128, 3, BL], F16, name="midT16")
            pattn = res.tile([128, B * H, L], F32, name="pattn")
            tb_all = res16.tile([NJR, CORES, B, 128], F16, name="tb_all")
            e3b = res16.tile([128, 2, BL], F16, name="e3b")
            # MM3 lhs tiles: per (chunk, b, r) a [NJRE, 128] f16 tile
            lt = {}
            for c in range(CORES):
                for b in range(B):
                    for r in range(NC):
                        lt[(c, b, r)] = res16.tile(
                            [NJRE, 128], F16, name=f"lt_{c}_{b}_{r}"
                        )
            # score staging in comb layout: [j', (b, r4, xl)], r>=2 cols stay 0
            score_ext = res.tile([128, B * 4 * XL], F32, name="score_ext")
            nc.vector.memset(score_ext[:], 0.0)

            # ================= stage A: MLPs + p_attn =================
            with (
                tc.tile_pool(name="mlpw", bufs=1) as mlpw,
                tc.tile_pool(name="psA", bufs=3, space="PSUM") as psA,
                tc.tile_pool(name="tmpA", bufs=2) as tmpA,
                tc.tile_pool(name="qpkp", bufs=1) as qpkp,
            ):
                teT_sb = mlpw.tile([128, 6, BL], F16)
                nc.sync.dma_start(teT_sb[:], teT16[:])

                # --- head / mid MLPs (full layer2) ---
                for nm, W2d, b2d, dst in (
                    ("h", Wh2, bh2, headT16),
                    ("m", Wm2, bm2, midT16),
                ):
                    W1d, b1d = mlp1[nm]
                    w1 = mlpw.tile([128, 6, HD], F16, name="w1s")
                    nc.sync.dma_start(w1[:], W1d[:])
                    b1 = mlpw.tile([128, 3], F32, name="b1s")
                    nc.sync.dma_start(b1[:], b1d[:])
                    w2 = mlpw.tile([128, 3, HD], F16, name="w2s")
                    nc.sync.dma_start(w2[:], W2d[:])
                    b2 = mlpw.tile([128, 3], F32, name="b2s")
                    nc.sync.dma_start(b2[:], b2d[:])

                    h1 = tmpA.tile([128, 3, BL], F16, name="h1")
                    for mt in range(3):
                        ps = psA.tile([128, BL], F32, name="psA")
                        for ks in range(6):
                            nc.tensor.matmul(
                                ps[:], w1[:, ks, mt * 128 : (mt + 1) * 128],
                                teT_sb[:, ks, :], start=(ks == 0), stop=(ks == 5),
                            )
                        nc.scalar.activation(
                            h1[:, mt, :], ps[:], A.Relu, bias=b1[:, mt : mt + 1]
                        )
                    for mt in range(3):
                        ps = psA.tile([128, BL], F32, name="psA")
                        for ks in range(3):
                            nc.tensor.matmul(
                                ps[:], w2[:, ks, mt * 128 : (mt + 1) * 128],
                                h1[:, ks, :], start=(ks == 0), stop=(ks == 2),
                            )
                        nc.scalar.activation(
                            dst[:, mt, :], ps[:], A.Identity, bias=b2[:, mt : mt + 1]
                        )
                    if debug:
                        key = "headT" if nm == "h" else "midT"
                        d32 = tmpA.tile([128, 3, BL], F32, name="d32")
                        for mt in range(3):
                            nc.vector.tensor_copy(d32[:, mt, :], dst[:, mt, :])
                        nc.sync.dma_start(dbg[key][:], d32[:])

                # --- tail MLP: full layer1; layer2 directly in MM3-lhs layout ---
                W1d, b1d = mlp1["t"]
                w1 = mlpw.tile([128, 6, HD], F16, name="w1s")
                nc.sync.dma_start(w1[:], W1d[:])
                b1 = mlpw.tile([128, 3], F32, name="b1s")
                nc.sync.dma_start(b1[:], b1d[:])
                w2b = mlpw.tile([128, 3, CORES, NC, NJRE], F16, name="w2b")
                nc.sync.dma_start(w2b[:], Wt2b[:])
                ltb = mlpw.tile([NJRE, CORES, NC], F32, name="ltb")
                nc.sync.dma_start(ltb[:], ltb_in[:])
                h1 = tmpA.tile([128, 3, BL], F16, name="h1")
                for mt in range(3):
                    ps = psA.tile([128, BL], F32, name="psA")
                    for ks in range(6):
                        nc.tensor.matmul(
                            ps[:], w1[:, ks, mt * 128 : (mt + 1) * 128],
                            teT_sb[:, ks, :], start=(ks == 0), stop=(ks == 5),
                        )
                    nc.scalar.activation(
                        h1[:, mt, :], ps[:], A.Relu, bias=b1[:, mt : mt + 1]
                    )
                for c in range(CORES):
                    for r in range(NC):
                        pst = psA.tile([NJRE, BL], F32, name="psA")
                        for ks in range(3):
                            nc.tensor.matmul(
                                pst[:], w2b[:, ks, c, r, :], h1[:, ks, :],
                                start=(ks == 0), stop=(ks == 2),
                            )
                        for b in range(B):
                            nc.scalar.activation(
                                lt[(c, b, r)][:],
                                pst[:, b * L : (b + 1) * L],
                                A.Identity,
                                bias=ltb[:, c, r : r + 1],
                            )
                if debug:
                    with tc.tile_pool(name="dbglt", bufs=2) as dbglt:
                        for c in range(CORES):
                            for b in range(B):
                                for r in range(NC):
                                    d32 = dbglt.tile([NJRE, 128], F32, name="dlt")
                                    nc.vector.tensor_copy(d32[:], lt[(c, b, r)][:])
                                    nc.sync.dma_start(dbg["lt"][c, b, r], d32[:])

                # --- p_attn ---
                wq_sb = mlpw.tile([128, 6, D], F16, name="wqk")
                nc.sync.dma_start(wq_sb[:], Wq[:])
                bq_sb = mlpw.tile([128, 6], F32, name="bqs")
                nc.sync.dma_start(bq_sb[:], bq[:])
                wk_sb = mlpw.tile([128, 6, D], F16, name="wqk")
                nc.sync.dma_start(wk_sb[:], Wk[:])
                bk_sb = mlpw.tile([128, 6], F32, name="bks")
                nc.sync.dma_start(bk_sb[:], bk[:])
                qT_sb = mlpw.tile([128, 6, BL], F16, name="qkT")
                nc.sync.dma_start(qT_sb[:], qryT[:])
                kT_sb = mlpw.tile([128, 6, BL], F16, name="qkT")
                nc.sync.dma_start(kT_sb[:], keyT[:])
                pm_sb = mlpw.tile([128, B, L], F32, name="pm")
                nc.sync.dma_start(pm_sb[:], pmask[:])

                qpT = qpkp.tile([128, 6, BL], F16, name="qpT")
                kpT = qpkp.tile([128, 6, BL], F16, name="kpT")
                for wmat, bvec, src, dst2 in (
                    (wq_sb, bq_sb, qT_sb, qpT),
                    (wk_sb, bk_sb, kT_sb, kpT),
                ):
                    for mt in range(6):
                        ps = psA.tile([128, BL], F32, name="psA")
                        for ks in range(6):
                            nc.tensor.matmul(
                                ps[:], wmat[:, ks, mt * 128 : (mt + 1) * 128],
                                src[:, ks, :], start=(ks == 0), stop=(ks == 5),
                            )
                        nc.scalar.activation(
                            dst2[:, mt, :], ps[:], A.Identity, bias=bvec[:, mt : mt + 1]
                        )

                inv_sqrt = 1.0 / math.sqrt(DK)
                for b in range(B):
                    for h in range(H):
                        ps = psA.tile([128, 128], F32, name="psA")
                        r0 = h * DK
                        segs = []
                        base = r0
                        while base < r0 + DK:
                            s_i, p0 = base // 128, base % 128
                            n = min(128 - p0, r0 + DK - base)
                            segs.append((s_i, p0, n))
                            base += n
                        for si, (s_i, p0, n) in enumerate(segs):
                            nc.tensor.matmul(
                                ps[:],
                                qpT[p0 : p0 + n, s_i, b * L : (b + 1) * L],
                                kpT[p0 : p0 + n, s_i, b * L : (b + 1) * L],
                                start=(si == 0), stop=(si == len(segs) - 1),
                            )
                        sc = tmpA.tile([128, 128], F32, name="scq")
                        nc.vector.scalar_tensor_tensor(
                            sc[:], ps[:], inv_sqrt, pm_sb[:, b, :], Alu.mult, Alu.add
                        )
                        mx = tmpA.tile([128, 1], F32, name="mxq")
                        nc.vector.tensor_reduce(mx[:], sc[:], Ax.X, Alu.max, negate=True)
                        esum = tmpA.tile([128, 1], F32, name="esq")
                        e = tmpA.tile([128, 128], F32, name="eq")
                        nc.scalar.activation(
                            e[:], sc[:], A.Exp, bias=mx[:], accum_out=esum[:]
                        )
                        rec = tmpA.tile([128, 1], F32, name="recq")
                        nc.vector.reciprocal(rec[:], esum[:])
                        nc.vector.tensor_scalar_mul(pattn[:, b * H + h, :], e[:], rec[:])
                if debug:
                    for b in range(B):
                        for h in range(H):
                            nc.sync.dma_start(dbg["pattn"][b, h], pattn[:, b * H + h, :])

            # ========== stage B: t_bias (all chunks) + corner ==========
            with (
                tc.tile_pool(name="sbB", bufs=1) as sbB,
                tc.tile_pool(name="psB", bufs=2, space="PSUM") as psB,
            ):
                wbi_sb = sbB.tile([128, 3, CORES, NJR], F16)
                nc.sync.dma_start(wbi_sb[:], Wbi[:])
                wcc_sb = sbB.tile([128, 3, 2], F16)
                nc.sync.dma_start(wcc_sb[:], Wcc[:])

                for c in range(CORES):
                    ps = psB.tile([NJR, BL], F32, name="psTB")
                    for ks in range(3):
                        nc.tensor.matmul(
                            ps[:], wbi_sb[:, ks, c, :], midT16[:, ks, :],
                            start=(ks == 0), stop=(ks == 2),
                        )
                    nc.scalar.activation(
                        tb_all[:, c, :, :],
                        ps[:].rearrange("p (b z) -> p b z", b=B),
                        A.Copy,
                    )
                if debug:
                    with tc.tile_pool(name="dbgtb", bufs=1) as dbgtb:
                        d32 = dbgtb.tile([NJR, CORES, B, 128], F32, name="dtb")
                        nc.vector.tensor_copy(d32[:], tb_all[:])
                        nc.sync.dma_start(dbg["tb"][:], d32[:])

                psc = psB.tile([2, BL], F32, name="psCC")
                for ks in range(3):
                    nc.tensor.matmul(
                        psc[:], wcc_sb[:, ks, :], midT16[:, ks, :],
                        start=(ks == 0), stop=(ks == 2),
                    )
                e3 = sbB.tile([2, BL], F16, name="e3")
                nc.scalar.activation(e3[:], psc[:], A.Copy, scale=0.125)
                # broadcast each r-row across partitions (DRAM bounce: SBUF
                # source APs may not have a zero partition step)
                nc.sync.dma_start(e3_dram[:], e3[:])
                for r in range(2):
                    nc.sync.dma_start(
                        e3b[:, r, :], e3_dram[r : r + 1, :].to_broadcast([128, BL])
                    )

            # ========== stage C: jr loop (MM1 + MM2 + corner-turned u write) =====
            with (
                tc.tile_pool(name="wchunk", bufs=2) as wchunk,
                tc.tile_pool(name="tbig", bufs=2) as tbigp,
                tc.tile_pool(name="psT", bufs=4, space="PSUM") as psT,
                tc.tile_pool(name="psU", bufs=2, space="PSUM") as psU,
                tc.tile_pool(name="ustg", bufs=3) as ustg,
            ):
                n_chunks = (NJRE + W_CHUNK - 1) // W_CHUNK
                for ch in range(n_chunks):
                    jr0 = ch * W_CHUNK
                    g = min(W_CHUNK, NJRE - jr0)
                    wt = wchunk.tile([128, 3, W_CHUNK, HD], F16, name="wt")
                    for s in range(3):
                        nc.sync.dma_start(
                            wt[:, s, :g, :],
                            W1c[jr0 : jr0 + g, s].rearrange("g k i -> k g i"),
                        )
                    # MM1 (f16): t_big[i, it, jl, (b z)]
                    t_big = tbigp.tile([128, 3, W_CHUNK, BL], F16, name="t_big")
                    for jl in range(g):
                        for it in range(3):
                            ps = psT.tile([128, BL], F32, name="psT")
                            for ks in range(3):
                                nc.tensor.matmul(
                                    ps[:],
                                    wt[:, ks, jl, it * 128 : (it + 1) * 128],
                                    midT16[:, ks, :],
                                    start=(ks == 0), stop=(ks == 2),
                                )
                            nc.scalar.activation(t_big[:, it, jl, :], ps[:], A.Copy)
                    # MM2 (f16): psu[x, (jl z)] per b -> staging [x, jl, b, z]
                    stg = ustg.tile([128, W_CHUNK, B, 128], F16, name="stg")
                    for b in range(B):
                        psu = psU.tile([128, W_CHUNK * 128], F32, name="psU")
                        rhs_n = g * 128
                        for it in range(3):
                            nc.tensor.matmul(
                                psu[:, :rhs_n],
                                headT16[:, it, b * L : (b + 1) * L],
                                t_big[:, it, :g, b * L : (b + 1) * L],
                                start=(it == 0), stop=(it == 2),
                            )
                        if jr0 < NJR:
                            nc.scalar.activation(
                                stg[:, :g, b, :],
                                psu[:, :rhs_n].rearrange("p (g z) -> p g z", z=128),
                                A.Copy,
                            )
                        else:
                            # bias-j columns: scale 1/8 and add corner term
                            nc.scalar.activation(
                                stg[:, :g, b, :],
                                psu[:, :rhs_n].rearrange("p (g z) -> p g z", z=128),
                                A.Copy, scale=0.125,
                            )
                            for rr in range(g):
                                nc.vector.tensor_tensor(
                                    stg[:, rr, b, :],
                                    stg[:, rr, b, :],
                                    e3b[:, rr, b * L : (b + 1) * L],
                                    Alu.add,
                                )
                    # corner-turned write: ua_in[dest, xl, jr, b, z] <- stg
                    if jr0 < JR_A:
                        udst = ua_in_a[:, :, jr0 : jr0 + g]
                    else:
                        udst = ua_in_b[:, :, jr0 - JR_A : jr0 - JR_A + g]
                    nc.sync.dma_start(
                        udst.rearrange("d xl jl b z -> (d xl) jl (b z)"),
                        stg[:, :g, :, :].rearrange("x j b z -> x j (b z)"),
                    )
                    if ch == (JR_A // W_CHUNK) - 1:
                        # first A2A covers chunks 0..11; overlaps chunks 12..24
                        nc.gpsimd.collective_compute(
                            "AllToAll",
                            Alu.bypass,
                            replica_groups=[list(range(CORES))],
                            ins=[ua_in_a[:]],
                            outs=[ua_out_a[:]],
                        )
                nc.gpsimd.collective_compute(
                    "AllToAll",
                    Alu.bypass,
                    replica_groups=[list(range(CORES))],
                    ins=[ua_in_b[:]],
                    outs=[ua_out_b[:]],
                )
            if debug:
                nc.sync.dma_start(dbg["ua"][:], ua_in_a[:])
                nc.sync.dma_start(dbg["ub"][:], ua_in_b[:])

            # ========== stage D: gather R, fold t_bias, MM3 + softmax + MM4 ======
            with (
                tc.tile_pool(name="Rp", bufs=1) as Rp,
                tc.tile_pool(name="psD", bufs=1, space="PSUM") as psD,
                tc.tile_pool(name="smp", bufs=2) as smp,
                tc.tile_pool(name="postp", bufs=4) as postp,
                tc.tile_pool(name="post16", bufs=4) as post16,
                tc.tile_pool(name="psE", bufs=2, space="PSUM") as psE,
                tc.tile_pool(name="ps4", bufs=1, space="PSUM") as ps4p,
            ):
                R_all = Rp.tile([NJRE, CORES, XL, B, 128], F16, name="R_all")
                for s in range(CORES):
                    nc.sync.dma_start(
                        R_all[0:JR_A, s].rearrange("j x b z -> j x (b z)"),
                        ua_out_a[s].rearrange("x j b z -> j x (b z)"),
                    )
                    nc.sync.dma_start(
                        R_all[JR_A:NJRE, s].rearrange("j x b z -> j x (b z)"),
                        ua_out_b[s].rearrange("x j b z -> j x (b z)"),
                    )
                # fold t_bias (broadcast over xl) into the tail-weighted rows
                for c in range(CORES):
                    nc.vector.tensor_tensor(
                        R_all[0:NJR, c],
                        R_all[0:NJR, c],
                        tb_all[:, c, None, :, :].broadcast_to([NJR, XL, B, 128]),
                        Alu.add,
                    )
                if debug:
                    nc.sync.dma_start(dbg["R"][:], R_all[:])

                for b in range(B):
                    for r in range(NC):
                        psd = psD.tile([128, XL * 128], F32, name="psD")
                        for c in range(CORES):
                            for q in range(XL // 4):
                                nc.tensor.matmul(
                                    psd[:, q * 512 : (q + 1) * 512],
                                    lt[(c, b, r)][:],
                                    R_all[:, c, 4 * q : 4 * q + 4, b, :],
                                    start=(c == 0), stop=(c == CORES - 1),
                                )
                        # masked evacuation: sm = s * m0 + madd  (psum -> SBUF)
                        sm_all = smp.tile([128, XL, 128], F32, name="sm_all")
                        nc.vector.tensor_tensor(
                            sm_all[:],
                            psd[:].rearrange("p (x z) -> p x z", z=128),
                            m0_sb[:], Alu.mult,
                        )
                        nc.vector.tensor_tensor(
                            sm_all[:], sm_all[:], madd_sb[:], Alu.add
                        )
                        if debug:
                            nc.sync.dma_start(dbg["s"][b, r], sm_all[:])

                        # softmax over z + MM4 + relu.Vw per xl tile
                        for xl in range(XL):
                            sm = sm_all[:, xl, :]
                            mx = postp.tile([128, 1], F32, name="mx")
                            nc.vector.tensor_reduce(
                                mx[:], sm, Ax.X, Alu.max, negate=True
                            )
                            e16 = post16.tile([128, 128], F16, name="e16")
                            esum = postp.tile([128, 1], F32, name="esum")
                            nc.scalar.activation(
                                e16[:], sm, A.Exp, bias=mx[:], accum_out=esum[:]
                            )
                            pse = psE.tile([128, 128], F16, name="psE")
                            nc.tensor.transpose(pse[:], e16[:], ident16[:])
                            eT = post16.tile([128, 128], F16, name="eT")
                            nc.scalar.activation(eT[:], pse[:], A.Copy)
                            ps4 = ps4p.tile([128, D], F32, name="ps4")
                            nc.tensor.matmul(
                                ps4[:, 0:512], eT[:], text_sb[:, b, 0:512],
                                start=True, stop=True,
                            )
                            nc.tensor.matmul(
                                ps4[:, 512:768], eT[:], text_sb[:, b, 512:768],
                                start=True, stop=True,
                            )
                            junk = post16.tile([128, D], F16, name="junk")
                            acc = postp.tile([128, 1], F32, name="acc")
                            nc.vector.scalar_tensor_tensor(
                                junk[:], ps4[:], 0.0, vwb[:, r, :],
                                Alu.max, Alu.mult, accum_out=acc[:],
                            )
                            rec = postp.tile([128, 1], F32, name="rec")
                            nc.vector.reciprocal(rec[:], esum[:])
                            col = b * 64 + xl * 4 + r
                            nc.vector.tensor_scalar(
                                score_ext[:, col : col + 1], acc[:],
                                rec[:], vbb[:, r : r + 1], Alu.mult, Alu.add,
                            )
                if debug:
                    nc.sync.dma_start(dbg["score"][:], score_ext[:])

                # scores -> comb layout:
                # ag_in[b, 4*xl + j//32, 4*(j%32) + r] = score_ext[j, (b, xl, r)]
                for b in range(B):
                    nc.sync.dma_start(
                        ag_in[b].rearrange("(xl a) (c r) -> (a c) xl r", a=4, r=4),
                        score_ext[:, b * 64 : (b + 1) * 64].rearrange(
                            "p (x r) -> p x r", r=4
                        ),
                    )

            nc.gpsimd.collective_compute(
                "AllGather",
                Alu.bypass,
                replica_groups=[list(range(CORES))],
                ins=[ag_in[:]],
                outs=[ag_sc[:]],
            )

            # ========== stage G: final combine (replicated) ==========
            with (
                tc.tile_pool(name="finp", bufs=4) as finp,
                tc.tile_pool(name="psF", bufs=2, space="PSUM") as psF,
            ):
                combs = {}
                mm = finp.tile([128, 2], F32, name="mm")  # col0 max, col1 -min
                first = True
                for b in range(B):
                    for h in range(H):
                        scx = finp.tile([128, 128], F32, name="scx")
                        nc.sync.dma_start(scx[0:64, :], ag_sc[2 * h, b])
                        nc.sync.dma_start(scx[64:128, :], ag_sc[2 * h + 1, b])
                        comb = finp.tile([128, 128], F32, name=f"comb_{b}_{h}")
                        nc.vector.tensor_tensor(
                            comb[:], pattn[:, b * H + h, :], scx[:], Alu.add
                        )
                        combs[(b, h)] = comb
                        if first:
                            nc.vector.tensor_reduce(
                                mm[:, 0:1], comb[:], Ax.X, Alu.max
                            )
                            nc.vector.tensor_reduce(
                                mm[:, 1:2], comb[:], Ax.X, Alu.min, negate=True
                            )
                            first = False
                        else:
                            t2 = finp.tile([128, 2], F32, name="t2")
                            nc.vector.tensor_reduce(t2[:, 0:1], comb[:], Ax.X, Alu.max)
                            nc.vector.tensor_reduce(
                                t2[:, 1:2], comb[:], Ax.X, Alu.min, negate=True
                            )
                            nc.vector.tensor_tensor(mm[:], mm[:], t2[:], Alu.max)
                # cross-partition: transpose [128, 2] -> [2, 128]
                psf = psF.tile([128, 128], F32, name="psF")
                nc.tensor.transpose(psf[0:2, :], mm[:], ident32[:])
                hilo = finp.tile([2, 128], F32, name="hilo")
                nc.vector.tensor_copy(hilo[:], psf[0:2, :])
                hl2 = finp.tile([2, 1], F32, name="hl2")
                nc.vector.tensor_reduce(hl2[:], hilo[:], Ax.X, Alu.max)
                # hi - lo = hl2[0] + hl2[1]: collapse partitions via DMA
                nc.sync.dma_start(hl_dram[:], hl2[:])
                hlrow = finp.tile([1, 2], F32, name="hlrow")
                nc.sync.dma_start(hlrow[:], hl_dram[:])
                rng = finp.tile([1, 1], F32, name="rng")
                nc.vector.tensor_reduce(rng[:], hlrow[:], Ax.X, Alu.add)
                rcp1 = finp.tile([1, 1], F32, name="rcp1")
                nc.vector.reciprocal(rcp1[:], rng[:])
                rcpb = finp.tile([128, 1], F32, name="rcpb")
                nc.sync.dma_start(rcp_dram[:], rcp1[:])
                nc.sync.dma_start(rcpb[:], rcp_dram[0:1, :].to_broadcast([128, 1]))

                for b in range(B):
                    for h in range(H):
                        comb = combs[(b, h)]
                        # softmax over y of comb * rcp  (shift by lo*rcp is a
                        # per-row constant -> softmax-invariant)
                        nrm = finp.tile([128, 128], F32, name="nrm")
                        nc.vector.tensor_scalar_mul(nrm[:], comb[:], rcpb[:])
                        mx = finp.tile([128, 1], F32, name="mxf")
                        nc.vector.tensor_reduce(
                            mx[:], nrm[:], Ax.X, Alu.max, negate=True
                        )
                        ef = finp.tile([128, 128], F32, name="ef")
                        esum = finp.tile([128, 1], F32, name="esf")
                        nc.scalar.activation(
                            ef[:], nrm[:], A.Exp, bias=mx[:], accum_out=esum[:]
                        )
                        rec = finp.tile([128, 1], F32, name="recf")
                        nc.vector.reciprocal(rec[:], esum[:])
                        of = finp.tile([128, 128], F32, name="of")
                        nc.vector.tensor_scalar_mul(of[:], ef[:], rec[:])
                        nc.sync.dma_start(out[b, h], of[:])

    _split_multiwaits(nc)
    return nc


# ----------------------------------------------------------------------------
# Host-side input preparation (per core)
# ----------------------------------------------------------------------------


def prep_inputs(inputs):
    """inputs: dict of full numpy arrays as produced by setup_inputs().
    Returns in_maps: list of per-core dicts."""
    f32, f16 = np.float32, np.float16
    te = np.ascontiguousarray(inputs["text_embeddings"], dtype=f32)  # [B, L, D]
    query = np.ascontiguousarray(inputs["query"], dtype=f32)
    key = np.ascontiguousarray(inputs["key"], dtype=f32)
    mask = inputs["mask"]
    Wtri = np.ascontiguousarray(inputs["Wtri"], dtype=f32)  # [385, 384, 385, 2]

    def kt(a, s, dt=f16):  # [K, M] -> [128, K//128, M]
        K, M = a.shape
        assert K == s * 128
        return np.ascontiguousarray(a.reshape(s, 128, M).transpose(1, 0, 2)).astype(dt)

    def rowsT(a, dt=f16):  # [B, L, D] -> [128, D//128, B*L] transposed k-tiled
        Dm = a.shape[-1]
        flat = a.reshape(-1, Dm).T  # [D, B*L]
        return np.ascontiguousarray(
            flat.reshape(Dm // 128, 128, flat.shape[1]).transpose(1, 0, 2)
        ).astype(dt)

    def bias_t(b, s):  # [s*128] -> [128, s]
        return np.ascontiguousarray(b.reshape(s, 128).T).astype(f32)

    common = {
        "teT16": rowsT(te),
        "text16": np.ascontiguousarray(te.transpose(1, 0, 2)).astype(f16),
        "Wh1": kt(inputs["Wh1"].astype(f32), 6),
        "bh1": bias_t(inputs["bh1"].astype(f32), 3),
        "Wh2": kt(inputs["Wh2"].astype(f32), 3),
        "bh2": bias_t(inputs["bh2"].astype(f32), 3),
        "Wm1": kt(inputs["Wm1"].astype(f32), 6),
        "bm1": bias_t(inputs["bm1"].astype(f32), 3),
        "Wm2": kt(inputs["Wm2"].astype(f32), 3),
        "bm2": bias_t(inputs["bm2"].astype(f32), 3),
        "Wt1": kt(inputs["Wt1"].astype(f32), 6),
        "bt1": bias_t(inputs["bt1"].astype(f32), 3),
        "Wq": kt(inputs["Wq"].astype(f32), 6),
        "bq": bias_t(inputs["bq"].astype(f32), 6),
        "Wk": kt(inputs["Wk"].astype(f32), 6),
        "bk": bias_t(inputs["bk"].astype(f32), 6),
        "qryT": rowsT(query),
        "keyT": rowsT(key),
        "pmask": np.ascontiguousarray(
            np.where(mask == 0, np.float32(-1e9), np.float32(0.0)).transpose(1, 0, 2)
        ),
        "Vw": inputs["Vw"].astype(f32).reshape(2, D),
        "Vb": inputs["Vb"].astype(f32).reshape(2, 1),
    }

    # tail layer-2 weights in MM3-lhs layout: [128, 3, 8 c, 2 r, 98]
    wt2 = inputs["Wt2"].astype(f32)  # [384, 384]
    bt2 = inputs["bt2"].astype(f32)  # [384]
    w2b = np.zeros((128, 3, CORES, NC, NJRE), dtype=f16)
    ltb = np.zeros((NJRE, CORES, NC), dtype=f32)
    for c in range(CORES):
        blk = wt2[:, c * JC : (c + 1) * JC].reshape(3, 128, JC).transpose(1, 0, 2)
        for r in range(NC):
            w2b[:, :, c, r, r * JC : (r + 1) * JC] = blk
            ltb[r * JC : (r + 1) * JC, c, r] = bt2[c * JC : (c + 1) * JC]
            ltb[NJR + r, c, r] = 1.0
    common["Wt2b"] = np.ascontiguousarray(w2b)
    common["ltb"] = np.ascontiguousarray(ltb)

    # t_bias rows for all chunks: [128, 3, 8, 96]
    wbi = np.empty((128, 3, CORES, NJR), dtype=f16)
    for c in range(CORES):
        bi = Wtri[384, :, c * JC : (c + 1) * JC, :]  # [384 k, 48 j, 2 r]
        tmp = bi.transpose(0, 2, 1).reshape(HD, NJR)  # [k, jr] r-major
        wbi[:, :, c, :] = tmp.reshape(3, 128, NJR).transpose(1, 0, 2)
    common["Wbi"] = np.ascontiguousarray(wbi)

    # corner [128, 3, 2]
    wcc = Wtri[384, :, 384, :]  # [384, 2]
    common["Wcc"] = np.ascontiguousarray(
        wcc.reshape(3, 128, 2).transpose(1, 0, 2)
    ).astype(f16)

    idx = np.arange(L)
    in_maps = []
    for c in range(CORES):
        m = dict(common)
        j0 = c * JC
        # W core slice -> [NJRE, 3, 128, 384]; jr = r*48 + jj
        blk = Wtri[:HD, :, j0 : j0 + JC, :]  # [384 i, 384 k, 48 j, 2 r]
        w1c = np.empty((NJRE, 3, 128, HD), dtype=f16)
        t = blk.transpose(3, 2, 1, 0)  # [r, j, k, i]
        w1c[:NJR] = t.reshape(NJR, 3, 128, HD)
        bj = Wtri[:HD, :, 384, :]  # [384 i, 384 k, 2 r]
        for r in range(2):
            w1c[NJR + r] = bj[:, :, r].T.reshape(3, 128, HD)
        m["W1c"] = np.ascontiguousarray(w1c)

        # softmax-z masks for this core's x chunk: [y(128), xl, z]
        xs = c * XL + np.arange(XL)
        zz = idx[None, None, :]
        yy = idx[:, None, None]
        xx = xs[None, :, None]
        bad = (zz > yy) | (zz < xx)  # [y, xl, z]
        m["m0"] = np.ascontiguousarray(np.where(bad, 0.0, 1.0).astype(f32))
        m["madd"] = np.ascontiguousarray(np.where(bad, -1e6, 0.0).astype(f32))
        in_maps.append(m)
    return in_maps


_CACHE = {}


def _get_built(debug=False):
    key = ("nc", debug)
    if key not in _CACHE:
        _CACHE[key] = build(debug=debug)
    return _CACHE[key]


def run(inputs, debug=False, trace=False):
    from concourse.bass_utils import run_bass_kernel_spmd

    nc = _get_built(debug=debug)
    in_maps = prep_inputs(inputs)
    res = run_bass_kernel_spmd(
        nc, in_maps, list(range(CORES)), trace=trace
    )
    return res


def kernel(**inputs):
    res = run(inputs, debug=False)
    return np.ascontiguousarray(res.results[0]["out"])


if __name__ == "__main__":
    nc = build(debug=False)
    print("build OK")
